# revision 1
# baseline (speedup 1.0000x reference)
"""Trainium2 Bass kernel for nn_CDC_62646392980082 (GRU-CPC loss_fn).

Contract: kernel(**inputs) takes the FULL unsharded inputs (numpy) and
returns the FULL output (loss, acc) exactly like the jax reference.

Strategy (8 NeuronCores, data-parallel over batch B=256 -> 32/core):
  - Transposed layouts (feature dims on SBUF partitions) so every
    contraction is a clean PE matmul; fp16 matmuls with fp32 PSUM
    accumulate and fp32 gate/softmax arithmetic.
  - Host pre-transposes weights/encodings once so all DMAs are
    contiguous; negatives are folded host-side into per-(prediction,
    cell) multiplicity counts so the random gather becomes dense masked
    reductions on the DVE.
  - Per-core partial sums of (loss, correct) are summed on host.
"""

import sys

if "/opt/trn_rl_repo" not in sys.path:
    sys.path.insert(0, "/opt/trn_rl_repo")

import numpy as np
import ml_dtypes

B, K, R, C, P, H, S = 256, 5, 6, 7, 1280, 256, 64
NCORE = 8
BS = B // NCORE            # 32 images per core
BC = BS * C                # 224 (b, c) columns
PC_N = P // 128            # 10 p-chunks
HC_N = H // 128            # 2 h-chunks
IJ = 49                    # 7x7 cells
PAIRS = [(k, r) for k in range(K) for r in range(R - k)]   # 20 valid (k, r)
NPAIR = len(PAIRS)
HALF = 10                  # pairs per pass
N_PREDS = NPAIR * B * C    # 35840 global predictions

_CACHE = {}


def _build_program():
    import concourse.bacc as bacc
    import concourse.mybir as mybir
    from concourse.tile import TileContext

    f32 = mybir.dt.float32
    bf16 = mybir.dt.float16  # fp16: same PE rate as bf16, 4x mantissa
    Alu = mybir.AluOpType
    Act = mybir.ActivationFunctionType

    nc = bacc.Bacc()
    dp = nc.declare_dram_parameter
    encT = dp("encT", [128, PC_N * R * BC], bf16, isOutput=False)   # GRU layout
    encB = dp("encB", [128, PC_N * BS * IJ], bf16, isOutput=False)  # dots layout
    wih = dp("wih", [128, PC_N * 768], bf16, isOutput=False)
    whh = dp("whh", [128, HC_N * 768], bf16, isOutput=False)
    wk = dp("wk", [K, 128, HC_N * P], bf16, isOutput=False)
    brz = dp("brz", [128, 4], f32, isOutput=False)
    bihn = dp("bihn", [128, 2], f32, isOutput=False)
    bhhn = dp("bhhn", [128, 2], f32, isOutput=False)
    wklo = dp("wklo", [128, K * PC_N], f32, isOutput=False)
    wkhi = dp("wkhi", [128, K * PC_N], f32, isOutput=False)
    corr = dp("corr", [70, 2 * BS * IJ], bf16, isOutput=False)
    cnt1 = dp("cnt1", [70, 2 * BS * IJ], bf16, isOutput=False)
    posm = dp("posm", [70, 2 * IJ], f32, isOutput=False)
    out = dp("out", [1, 2], f32, isOutput=True)

    with TileContext(nc, pool_alloc_mode="queue") as tc:
        with tc.tile_pool(name="pers", bufs=1) as pers:
            # ---- persistent small loads ----
            brz_t = pers.tile([128, 4], f32)
            nc.sync.dma_start(out=brz_t, in_=brz[:, :])
            bihn_t = pers.tile([128, 2], f32)
            nc.sync.dma_start(out=bihn_t, in_=bihn[:, :])
            bhhn_t = pers.tile([128, 2], f32)
            nc.sync.dma_start(out=bhhn_t, in_=bhhn[:, :])
            wklo_t = pers.tile([128, K * PC_N], f32)
            nc.sync.dma_start(out=wklo_t, in_=wklo[:, :])
            wkhi_t = pers.tile([128, K * PC_N], f32)
            nc.sync.dma_start(out=wkhi_t, in_=wkhi[:, :])
            whh_b = pers.tile([128, HC_N * 768], bf16, name="whh_b")
            nc.sync.dma_start(out=whh_b, in_=whh[:, :])
            whh_t = [whh_b[:, h * 768 : (h + 1) * 768] for h in range(HC_N)]

            # zero initial hidden state (bf16)
            zb = pers.tile([128, 256], bf16)
            nc.vector.memset(zb, 0.0)

            # GRU context: per-(h-chunk, r-pair) tiles [128, 512] bf16;
            # each r block is 256 cols = 224 real + 32 pad (zeroed)
            ctxp = [
                [pers.tile([128, 512], bf16, tag=f"ctx{h}_{rp}", name=f"ctx{h}_{rp}") for rp in range(R // 2)]
                for h in range(HC_N)
            ]
            for h in range(HC_N):
                for rp in range(R // 2):
                    pv = ctxp[h][rp].rearrange("p (q x) -> p q x", q=2)[:, :, BC:]
                    nc.vector.memset(pv, 0.0)

            def ctx_r(h, r):
                return ctxp[h][r // 2][:, (r % 2) * 256 : (r % 2) * 256 + 256]

            outS = pers.tile([1, 2], f32)
            # gi chunks of 3 steps each (672 cols)
            GI_CH = [(0, 672), (672, 672)]

            gis = [
                [pers.tile([128, w], f32, tag=f"gis{m}_{c}", name=f"gis{m}_{c}") for c, (o, w) in enumerate(GI_CH)]
                for m in range(6)
            ]

            def gi_slice(m, r):
                ci, rem = divmod(r, 3)
                return gis[m][ci][:, rem * BC : (rem + 1) * BC]

            # preds-side pool opened early so preds can interleave with GRU
            ppA = tc.alloc_tile_pool(name="ppA", bufs=1)
            psPP = tc.alloc_tile_pool(name="psPP", bufs=3, space="PSUM")
            psGH = tc.alloc_tile_pool(name="psGH", bufs=3, space="PSUM")
            predsT = [
                ppA.tile([128, BS * HALF * C], bf16, tag=f"pt{i}", name=f"pt{i}")
                for i in range(PC_N)
            ]

            def emit_wk(pass_i, k):
                wkb_big = ppA.tile(
                    [128, HC_N * P], bf16, tag="wkbig", bufs=2,
                    name=f"wk{pass_i}_{k}",
                )
                for s in range(2):
                    sl = slice(64 * s, 64 * s + 64)
                    nc.sync.dma_start(out=wkb_big[sl, :], in_=wk[k, sl, :])
                return [wkb_big[:, hc * P : (hc + 1) * P] for hc in range(HC_N)]

            def emit_preds_chunk(pass_i, k, wk_t, qc, nq, rs):
                for m in range(PC_N):
                    ps = psPP.tile(
                        [128, 512], f32, tag="pp", name=f"pp_{pass_i}_{qc}_{m}"
                    )
                    for hc in range(HC_N):
                        if nq == 2:
                            assert rs[1] == rs[0] + 1 and rs[0] % 2 == 0
                            rhs = ctxp[hc][rs[0] // 2]
                        else:
                            rhs = ctx_r(hc, rs[0])
                        nc.tensor.matmul(
                            ps[:, : nq * 256],
                            wk_t[hc][:, m * 128 : (m + 1) * 128],
                            rhs,
                            start=(hc == 0),
                            stop=(hc == HC_N - 1),
                        )
                    psv = ps.rearrange("p (q x) -> p q x", q=2)[
                        :, :nq, :BC
                    ].rearrange("p q (b c) -> p q b c", b=BS)
                    dst = predsT[m].rearrange(
                        "p (b q c) -> p q b c", b=BS, q=HALF
                    )[:, qc : qc + nq, :, :]
                    lo = wklo_t[:, k * PC_N + m : k * PC_N + m + 1]
                    hi = wkhi_t[:, k * PC_N + m : k * PC_N + m + 1]
                    if m % 2 == 0:
                        nc.vector.tensor_scalar(dst, psv, lo, hi, Alu.max, Alu.min)
                    else:
                        at = ppA.tile(
                            [128, nq * BC], f32, tag="at", bufs=2,
                            name=f"at_{pass_i}_{qc}_{m}",
                        )
                        atv = at.rearrange("p (q x) -> p q x", q=nq)
                        nc.scalar.activation(
                            atv,
                            ps.rearrange("p (q x) -> p q x", q=2)[:, :nq, :BC],
                            Act.Identity,
                        )
                        nc.vector.tensor_scalar(
                            dst,
                            at.rearrange("p (q b c) -> p q b c", q=nq, b=BS),
                            lo, hi, Alu.max, Alu.min,
                        )

            def preds_runs(pass_i):
                ppairs = PAIRS[pass_i * HALF : (pass_i + 1) * HALF]
                runs = []
                q = 0
                while q < HALF:
                    k = ppairs[q][0]
                    q0 = q
                    while q < HALF and ppairs[q][0] == k:
                        q += 1
                    runs.append((k, q0, q))
                return ppairs, runs

            def emit_preds_run(pass_i, k, q0, q1, ppairs, wk_t=None):
                if wk_t is None:
                    wk_t = emit_wk(pass_i, k)
                for qc in range(q0, q1, 2):
                    nq = min(2, q1 - qc)
                    rs = [ppairs[qc + i][1] for i in range(nq)]
                    emit_preds_chunk(pass_i, k, wk_t, qc, nq, rs)

            # ---- phase 1: gi = x @ W_ih.T, interleaved with GRU steps ----
            with (
                tc.tile_pool(name="p1", bufs=1) as p1,
                tc.tile_pool(name="psGI", bufs=2, space="PSUM") as psGI,
            ):
                enc_b = p1.tile([128, PC_N * R * BC], bf16, name="enc_b")
                wih_b = p1.tile([128, PC_N * 768], bf16, name="wih_b")
                for i in range(PC_N):   # pc-ordered column DMAs (match consumption)
                    csl = slice(i * R * BC, (i + 1) * R * BC)
                    wsl = slice(i * 768, (i + 1) * 768)
                    nc.sync.dma_start(out=wih_b[:, wsl], in_=wih[:, wsl])
                    nc.sync.dma_start(out=enc_b[:, csl], in_=encT[:, csl])
                enc_t = [enc_b[:, i * R * BC : (i + 1) * R * BC] for i in range(PC_N)]
                wih_t = [wih_b[:, i * 768 : (i + 1) * 768] for i in range(PC_N)]

                def emit_gi_chunk(ci, kh):
                    # kh 0: contract pc 0-4 (copy out); kh 1: pc 5-9 (add in)
                    off, w = GI_CH[ci]
                    pcs = range(5 * kh, 5 * kh + 5)
                    for m in range(6):
                        ps = psGI.tile([128, 512], f32, tag="gi", name=f"gi_{ci}_{m}_{kh}")
                        for h2 in range(2):          # 672 = 336+336 (<=512 psum)
                            lo, wd = h2 * 336, 336
                            for pc in pcs:
                                nc.tensor.matmul(
                                    ps[:, :wd],
                                    wih_t[pc][:, m * 128 : (m + 1) * 128],
                                    enc_t[pc][:, off + lo : off + lo + wd],
                                    start=(pc == pcs[0]),
                                    stop=(pc == pcs[-1]),
                                )
                            gslice = gis[m][ci][:, lo : lo + wd]
                            if kh == 0:
                                nc.vector.tensor_copy(gslice, ps[:, :wd])
                            else:
                                nc.vector.tensor_tensor(
                                    gslice, gslice, ps[:, :wd], op=Alu.add
                                )

                def emit_gru_step(r):
                    hprev = [zb, zb] if r == 0 else [ctx_r(h, r - 1) for h in range(HC_N)]
                    ghp = []
                    for m in range(6):
                        ps = psGH.tile([128, 256], f32, tag="gh", name=f"gh_{r}_{m}")
                        for hc in range(HC_N):
                            nc.tensor.matmul(
                                ps,
                                whh_t[hc][:, m * 128 : (m + 1) * 128],
                                hprev[hc],
                                start=(hc == 0),
                                stop=(hc == HC_N - 1),
                            )
                        ghp.append(ps)
                    for t in range(2):
                        iR = gi_slice(0 + t, r)
                        iZ = gi_slice(2 + t, r)
                        iN = gi_slice(4 + t, r)
                        hR = ghp[0 + t][:, :BC]
                        hZ = ghp[2 + t][:, :BC]
                        hN = ghp[4 + t][:, :BC]
                        tA = pers.tile([128, BC], f32, tag="tA", bufs=2, name=f"tA{r}{t}")
                        nc.vector.tensor_tensor(tA, iR, hR, op=Alu.add)
                        rt = pers.tile([128, BC], f32, tag="rt", bufs=2, name=f"rt{r}{t}")
                        nc.scalar.activation(rt, tA, Act.Sigmoid, bias=brz_t[:, 0 + t : 1 + t])
                        tB = pers.tile([128, BC], f32, tag="tB", bufs=2, name=f"tB{r}{t}")
                        nc.vector.tensor_tensor(tB, iZ, hZ, op=Alu.add)
                        zt = pers.tile([128, BC], f32, tag="zt", bufs=2, name=f"zt{r}{t}")
                        nc.scalar.activation(zt, tB, Act.Sigmoid, bias=brz_t[:, 2 + t : 3 + t])
                        tV = pers.tile([128, BC], f32, tag="tV", bufs=2, name=f"tV{r}{t}")
                        nc.vector.scalar_tensor_tensor(
                            tV, hN, bhhn_t[:, t : t + 1], rt, op0=Alu.add, op1=Alu.mult
                        )
                        tW = pers.tile([128, BC], f32, tag="tW", bufs=2, name=f"tW{r}{t}")
                        nc.vector.tensor_tensor(tW, tV, iN, op=Alu.add)
                        nt = pers.tile([128, BC], f32, tag="nt", bufs=2, name=f"nt{r}{t}")
                        nc.scalar.activation(nt, tW, Act.Tanh, bias=bihn_t[:, t : t + 1])
                        tD = pers.tile([128, BC], f32, tag="tD", bufs=2, name=f"tD{r}{t}")
                        nc.vector.tensor_tensor(tD, hprev[t][:, :BC], nt, op=Alu.subtract)
                        tE = pers.tile([128, BC], f32, tag="tE", bufs=2, name=f"tE{r}{t}")
                        nc.vector.tensor_tensor(tE, zt, tD, op=Alu.mult)
                        hout = ctx_r(t, r)[:, :BC]
                        nc.vector.tensor_tensor(hout, nt, tE, op=Alu.add)

                # interleave emission: gi tail and preds(k=0) overlap the GRU
                pp0, runs0 = preds_runs(0)
                emit_gi_chunk(0, 0)
                emit_gi_chunk(1, 0)
                emit_gi_chunk(0, 1)
                emit_gru_step(0)
                emit_gi_chunk(1, 1)
                wk0 = emit_wk(0, 0)
                emit_gru_step(1)
                emit_preds_chunk(0, 0, wk0, 0, 2, [0, 1])
                emit_gru_step(2)
                emit_gru_step(3)
                emit_preds_chunk(0, 0, wk0, 2, 2, [2, 3])
                emit_gru_step(4)
                emit_gru_step(5)
                emit_preds_chunk(0, 0, wk0, 4, 2, [4, 5])
                # rest of pass 0 (k=1)
                for k, q0, q1 in runs0[1:]:
                    emit_preds_run(0, k, q0, q1, pp0)

            psGH.release()

            # ---- phase 3: dots + loss (pass-1 preds emitted here too) ----
            with (
                tc.tile_pool(name="pp", bufs=1) as ppool,
                tc.tile_pool(name="psDP", bufs=5, space="PSUM") as psDP,
            ):
                encB_b = ppool.tile([128, PC_N * BS * IJ], bf16, name="encB_b")
                for s in range(4):
                    sl = slice(32 * s, 32 * s + 32)
                    eng = nc.sync if s % 2 == 0 else nc.gpsimd
                    eng.dma_start(out=encB_b[sl, :], in_=encB[sl, :])
                encB_t = [encB_b[:, i * BS * IJ : (i + 1) * BS * IJ] for i in range(PC_N)]
                posm_t = ppool.tile([70, 2 * IJ], f32)
                nc.sync.dma_start(out=posm_t, in_=posm[:, :])
                cnt1_t = ppool.tile([70, 2 * BS * IJ], bf16)
                nc.sync.dma_start(out=cnt1_t, in_=cnt1[:, :])
                corr_t = ppool.tile([70, 2 * BS * IJ], bf16)
                nc.sync.dma_start(out=corr_t, in_=corr[:, :])
                D = ppool.tile([70, 2 * BS * IJ], f32)
                B2 = ppool.tile([70, 2 * BS * IJ], f32)
                G2 = BS  # groups per half
                mx = ppool.tile([70, 2 * G2], f32, tag="mx")
                se = ppool.tile([70, 2 * G2], f32, tag="se")
                pos = ppool.tile([70, 2 * G2], f32, tag="pos")
                lnv = ppool.tile([70, 2 * G2], f32, tag="lnv")
                corr = ppool.tile([70, 2 * G2], f32, tag="corr")
                Ssum = ppool.tile([70, 8], f32, tag="S")

                def emit_dots_pass(pass_i, b0=0, b1=BS):
                    for b in range(b0, b1):
                        ps = psDP.tile([70, IJ], f32, tag="dp", name=f"dp{pass_i}_{b}")
                        for pc in range(PC_N):
                            nc.tensor.matmul(
                                ps,
                                predsT[pc][:, b * 70 : (b + 1) * 70],
                                encB_t[pc][:, b * IJ : (b + 1) * IJ],
                                start=(pc == 0),
                                stop=(pc == PC_N - 1),
                            )
                        gsl = slice(
                            (pass_i * BS + b) * IJ, (pass_i * BS + b + 1) * IJ
                        )
                        nc.vector.tensor_tensor(D[:, gsl], ps, corr_t[:, gsl], op=Alu.add)

                PG = 16  # groups per post part (4 parts)

                def emit_post_part(pi):
                    h = pi // 2
                    lo = pi * PG * IJ
                    hi = (pi + 1) * PG * IJ
                    Dh = D[:, lo:hi]
                    B2h = B2[:, lo:hi]
                    Dv = Dh.rearrange("p (g j) -> p g j", j=IJ)
                    B2v = B2h.rearrange("p (g j) -> p g j", j=IJ)
                    cnt_h = cnt1_t[:, lo:hi]
                    gsl = slice(pi * PG, (pi + 1) * PG)
                    mxh = mx[:, gsl]
                    seh = se[:, gsl]
                    posh = pos[:, gsl]
                    lnvh = lnv[:, gsl]
                    corrh = corr[:, gsl]
                    nc.vector.tensor_scalar(B2h, cnt_h, 0.0, -1e30, Alu.is_equal, Alu.mult)
                    nc.vector.tensor_tensor(Dh, Dh, B2h, op=Alu.add)
                    nc.vector.tensor_reduce(mxh, Dv, axis=mybir.AxisListType.X, op=Alu.max)
                    nc.vector.tensor_tensor(
                        B2v, Dv, mxh.unsqueeze(2).broadcast_to([70, PG, IJ]), op=Alu.subtract
                    )
                    nc.scalar.activation(B2h, B2h, Act.Exp)
                    nc.vector.tensor_tensor(B2h, B2h, cnt_h, op=Alu.mult)
                    nc.vector.tensor_reduce(seh, B2v, axis=mybir.AxisListType.X, op=Alu.add)
                    # pos = sum(D * posmask) (exact: zeros elsewhere)
                    pmh = posm_t[:, h * IJ : (h + 1) * IJ]
                    nc.vector.tensor_tensor(
                        B2v, Dv, pmh.unsqueeze(1).broadcast_to([70, PG, IJ]), op=Alu.mult
                    )
                    nc.vector.tensor_reduce(posh, B2v, axis=mybir.AxisListType.X, op=Alu.add)
                    # loss = ln(se) + mx - pos ; correct = (pos >= mx)
                    nc.scalar.activation(lnvh, seh, Act.Ln)
                    nc.vector.tensor_tensor(lnvh, lnvh, mxh, op=Alu.add)
                    nc.vector.tensor_tensor(corrh, posh, mxh, op=Alu.is_ge)
                    nc.vector.tensor_tensor(lnvh, lnvh, posh, op=Alu.subtract)
                    nc.vector.tensor_reduce(
                        Ssum[:, 2 * pi : 2 * pi + 1], lnvh,
                        axis=mybir.AxisListType.X, op=Alu.add,
                    )
                    nc.vector.tensor_reduce(
                        Ssum[:, 2 * pi + 1 : 2 * pi + 2], corrh,
                        axis=mybir.AxisListType.X, op=Alu.add,
                    )

                emit_dots_pass(0)
                pp1, runs1 = preds_runs(1)
                for k, q0, q1 in runs1:
                    emit_preds_run(1, k, q0, q1, pp1)
                emit_post_part(0)
                emit_post_part(1)
                emit_dots_pass(1, 0, 16)
                emit_post_part(2)
                emit_dots_pass(1, 16, 32)
                emit_post_part(3)

                # combine quarters: [loss, acc] = colsums of Ssum pairs
                ones = ppool.tile([70, 1], f32, tag="ones")
                nc.vector.memset(ones, 1.0)
                fp = psDP.tile([1, 8], f32, tag="dp", name="fin")
                nc.tensor.matmul(fp, ones, Ssum, start=True, stop=True)
                fs = ppool.tile([1, 8], f32, tag="fs")
                nc.vector.tensor_copy(fs, fp)
                fs2 = ppool.tile([1, 4], f32, tag="fs2")
                nc.vector.tensor_tensor(fs2[:, 0:2], fs[:, 0:2], fs[:, 2:4], op=Alu.add)
                nc.vector.tensor_tensor(fs2[:, 2:4], fs[:, 4:6], fs[:, 6:8], op=Alu.add)
                nc.vector.tensor_tensor(outS, fs2[:, 0:2], fs2[:, 2:4], op=Alu.add)
                nc.sync.dma_start(out=out[:, :], in_=outS)
            psPP.release()
            ppA.release()

    nc.finalize()
    return nc


def _prep_inputs(encodings, hidden, W_ih, W_hh, b_ih, b_hh, Wk_w, Wk_b,
                 neg_rows, neg_cols):
    """Host-side reformat of the full inputs into per-core DMA-clean arrays."""
    bf16 = np.float16
    enc = np.ascontiguousarray(encodings, dtype=np.float32)
    e6 = enc.reshape(NCORE, BS, C, C, PC_N, 128)  # (core, b, i, c, pc, pp)
    # GRU layout: [core, pc, pp, r*BC + b*7 + c], r < 6
    encT = np.ascontiguousarray(
        e6[:, :, :R].transpose(0, 5, 4, 2, 1, 3)   # (core, pp, pc, r, b, c)
    ).reshape(NCORE, 128, PC_N * R * BC).astype(bf16)
    # dots layout: [core, pc, pp, b*49 + i*7 + c]
    encB = np.ascontiguousarray(
        e6.transpose(0, 5, 4, 1, 2, 3)   # (core, pp, pc, b, i, c)
    ).reshape(NCORE, 128, PC_N * BS * IJ).astype(bf16)

    wih = np.ascontiguousarray(
        W_ih.T.reshape(PC_N, 128, 768).transpose(1, 0, 2), dtype=np.float32
    ).reshape(128, PC_N * 768).astype(bf16)
    whh = np.ascontiguousarray(
        W_hh.T.reshape(HC_N, 128, 768).transpose(1, 0, 2), dtype=np.float32
    ).reshape(128, HC_N * 768).astype(bf16)
    wkh = np.ascontiguousarray(
        Wk_w.transpose(0, 2, 1).reshape(K, HC_N, 128, P).transpose(0, 2, 1, 3),
        dtype=np.float32,
    ).reshape(K, 128, HC_N * P).astype(bf16)
    bsum = (b_ih + b_hh).astype(np.float32)
    brz = np.ascontiguousarray(bsum[:512].reshape(4, 128).T)
    bihn = np.ascontiguousarray(b_ih[512:].astype(np.float32).reshape(2, 128).T)
    bhhn = np.ascontiguousarray(b_hh[512:].astype(np.float32).reshape(2, 128).T)
    wkbT = np.ascontiguousarray(
        Wk_b.astype(np.float32).reshape(K, PC_N, 128).transpose(2, 0, 1)
    ).reshape(128, K * PC_N)
    wklo = -1.0 - wkbT
    wkhi = 1.0 - wkbT
    # rank-1 bias correction: corr[k, b, ij] = sum_p Wk_b[k,p] * enc[b,i,j,p]
    corr_k = np.einsum(
        "kp,bijp->kbij", Wk_b.astype(np.float32), enc, optimize=True
    ).reshape(K, B, IJ)
    # expand to device layout [core, row=q*7+c, half, b_local, j] (k by pair)
    corr_dev = np.empty((NCORE, HALF * C, 2, BS, IJ), dtype=np.float32)
    for half in range(2):
        for qq in range(HALF):
            k, _r = PAIRS[half * HALF + qq]
            for c in range(C):
                corr_dev[:, qq * 7 + c, half] = corr_k[k].reshape(NCORE, BS, IJ)
    corr_dev = corr_dev.reshape(NCORE, HALF * C, 2 * BS * IJ).astype(bf16)

    # negatives -> multiplicity counts over the 49 cells, plus the positive
    neg_idx = (neg_rows.astype(np.int64) * 7 + neg_cols.astype(np.int64))  # [B,K,R,C,63]
    sel = np.stack([neg_idx[:, k, r] for (k, r) in PAIRS], axis=1)  # [B,20,C,63]
    flat = (
        np.arange(B * NPAIR * C, dtype=np.int64)[:, None] * IJ
        + sel.reshape(B * NPAIR * C, S - 1)
    ).ravel()
    cnts = np.bincount(flat, minlength=B * NPAIR * C * IJ).reshape(
        B, NPAIR, C, IJ
    ).astype(np.float32)
    cvec = np.arange(C)
    for pi, (k, r) in enumerate(PAIRS):
        cnts[:, pi, cvec, r * 7 + cvec] += 1.0   # include the positive
    # device layout [core, row=q*7+c, half, b_local, j]
    cnt1 = np.ascontiguousarray(
        cnts.reshape(NCORE, BS, 2, HALF, C, IJ).transpose(0, 3, 4, 2, 1, 5)
    ).reshape(NCORE, HALF * C, 2 * BS * IJ).astype(bf16)

    posm = np.zeros((HALF * C, 2, IJ), dtype=np.float32)
    for half in range(2):
        for qq in range(HALF):
            k, r = PAIRS[half * HALF + qq]
            for c in range(C):
                posm[qq * 7 + c, half, r * 7 + c] = 1.0
    posm = posm.reshape(HALF * C, 2 * IJ)

    in_maps = []
    for core in range(NCORE):
        in_maps.append(
            {
                "encT": encT[core],
                "encB": encB[core],
                "wih": wih,
                "whh": whh,
                "wk": wkh,
                "brz": brz,
                "bihn": bihn,
                "bhhn": bhhn,
                "wklo": wklo,
                "wkhi": wkhi,
                "corr": corr_dev[core],
                "cnt1": cnt1[core],
                "posm": posm,
            }
        )
    return in_maps


def _get_program():
    if "nc" not in _CACHE:
        _CACHE["nc"] = _build_program()
    return _CACHE["nc"]


def run_on_device(in_maps, trace=False, tmpdir=None):
    from concourse.bass_utils import run_bass_kernel_spmd

    nc = _get_program()
    return run_bass_kernel_spmd(
        nc, in_maps, list(range(NCORE)), trace=trace, tmpdir=tmpdir
    )


def kernel(**inputs):
    in_maps = _prep_inputs(**inputs)
    res = run_on_device(in_maps)
    loss_sum = 0.0
    corr_sum = 0.0
    for core in range(NCORE):
        o = res.results[core]["out"]
        loss_sum += float(o[0, 0])
        corr_sum += float(o[0, 1])
    loss = np.float32(loss_sum / N_PREDS)
    acc = np.float32(corr_sum / N_PREDS)
    return loss, acc



# revision 11
# speedup vs baseline: 1.2034x; 1.2034x over previous
"""Trainium2 Bass kernel for nn_CDC_62646392980082 (GRU-CPC loss_fn), v2.

Contract: kernel(**inputs) takes the FULL unsharded inputs (numpy) and
returns the FULL output (loss, acc) exactly like the jax reference.

Strategy (8 NeuronCores, data-parallel over batch B=256 -> 32/core):
  - GRU gates fused in PSUM: gi (x@W_ih) and gh (h@W_hh) accumulate into
    the same PSUM bank per step; sigmoid reads PSUM directly on the
    scalar engine (no gi copies / adds on the DVE).
  - H' = h+1 reparameterization: h' = (n+1)(1-z) + z*h' with n+1 =
    2*sigmoid(2x); rank-1 corrections folded into biases host-side.
    Avoids tanh table swaps and one DVE op per gate step.
  - preds split into two passes by r: pass0 = {k2:r0-3, k3:r0-2,
    k4:r0-1} (63 rows), pass1 = {k0:r0-5, k1:r0-4} (77 rows), so
    pass0's dots can overlap the preds tail.
  - clip alternates engines per p-chunk: even chunks clip directly from
    PSUM on the DVE; odd chunks evacuate via scalar-engine Identity and
    clip fp16->fp16 on the DVE fast path.
  - negatives folded host-side into multiplicity counts; the cnt==0
    mask (-60000) is folded into the corr tensor; softmax shift uses a
    per-partition-row max so exp's bias port applies it for free.
"""

import sys

if "/opt/trn_rl_repo" not in sys.path:
    sys.path.insert(0, "/opt/trn_rl_repo")

import numpy as np

B, K, R, C, P, H, S = 256, 5, 6, 7, 1280, 256, 64
NCORE = 8
BS = B // NCORE            # 32 images per core
BC = BS * C                # 224 (b, c) columns
PC_N = P // 128            # 10 p-chunks
HC_N = H // 128            # 2 h-chunks
IJ = 49                    # 7x7 cells

# pass structure: pairs (k, r) grouped so pass0 finishes by GRU step 3
PASS_PAIRS = [
    [(2, 0), (2, 1), (2, 2), (2, 3), (3, 0), (3, 1), (3, 2), (4, 0), (4, 1)],
    [(0, 0), (0, 1), (0, 2), (0, 3), (0, 4), (0, 5),
     (1, 0), (1, 1), (1, 2), (1, 3), (1, 4)],
]
PR = [len(PASS_PAIRS[0]) * C, len(PASS_PAIRS[1]) * C]   # 63, 77 rows
ROW_OFF = {}
for _pi, _lst in enumerate(PASS_PAIRS):
    for _qi, _kr in enumerate(_lst):
        ROW_OFF[_kr] = (_pi, _qi * C)

# preds chunks: (pass, k, [r...]) with adjacent r, emitted after step max(r)
CHUNKS = [
    (0, 4, [0, 1]), (0, 3, [0, 1]),            # ready after step 1
    (0, 3, [2]), (1, 1, [0, 1]),               # after step 2
    (0, 2, [0, 1]), (0, 2, [2, 3]),
    (1, 0, [0, 1]), (1, 0, [2, 3]), (1, 1, [2, 3]),   # after step 3
    (1, 1, [4]),                               # after step 4
    (1, 0, [4, 5]),                            # after step 5
]
N_PREDS = 20 * B * C       # 35840 global predictions
MASK = np.float32(-60000.0)

_CACHE = {}


def _build_program():
    import concourse.bacc as bacc
    import concourse.mybir as mybir
    from concourse.tile import TileContext

    f32 = mybir.dt.float32
    f16 = mybir.dt.float16
    Alu = mybir.AluOpType
    Act = mybir.ActivationFunctionType
    AxX = mybir.AxisListType.X

    nc = bacc.Bacc()
    dp = nc.declare_dram_parameter
    encT = dp("encT", [128, R * PC_N * BC], f16, isOutput=False)   # r-major
    encB = dp("encB", [128, PC_N * BS * IJ], f16, isOutput=False)
    wih = dp("wih", [128, PC_N * 768], f16, isOutput=False)
    whh = dp("whh", [128, HC_N * 768], f16, isOutput=False)
    wk = dp("wk", [K, 128, HC_N * P], f16, isOutput=False)
    brz = dp("brz", [128, 4], f32, isOutput=False)     # r/z bias (H'-folded)
    nbrz = dp("nbrz", [128, 2], f32, isOutput=False)   # negated z bias
    bhn = dp("bhn", [128, 2], f32, isOutput=False)     # h-side n bias
    bin_ = dp("bin", [128, 2], f32, isOutput=False)    # x-side n bias
    wklo = dp("wklo", [128, K * PC_N], f32, isOutput=False)
    wkhi = dp("wkhi", [128, K * PC_N], f32, isOutput=False)
    corr0 = dp("corr0", [PR[0], BS * IJ], f16, isOutput=False)  # corr - mask
    corr1 = dp("corr1", [PR[1], BS * IJ], f16, isOutput=False)
    cnt0 = dp("cnt0", [PR[0], BS * IJ], f16, isOutput=False)
    cnt1 = dp("cnt1", [PR[1], BS * IJ], f16, isOutput=False)
    posm0 = dp("posm0", [PR[0], IJ], f16, isOutput=False)
    posm1 = dp("posm1", [PR[1], IJ], f16, isOutput=False)
    out = dp("out", [1, 8], f32, isOutput=True)
    corr_d = [corr0, corr1]
    cnt_d = [cnt0, cnt1]
    posm_d = [posm0, posm1]

    with TileContext(nc, pool_alloc_mode="queue") as tc:
        with tc.tile_pool(name="pers", bufs=1) as pers:
            # ---- persistent small loads (sync queue) ----
            brz_t = pers.tile([128, 4], f32)
            nc.sync.dma_start(out=brz_t, in_=brz[:, :])
            nbrz_t = pers.tile([128, 2], f32)
            nc.sync.dma_start(out=nbrz_t, in_=nbrz[:, :])
            bhn_t = pers.tile([128, 2], f32)
            nc.sync.dma_start(out=bhn_t, in_=bhn[:, :])
            bin_t = pers.tile([128, 2], f32)
            nc.sync.dma_start(out=bin_t, in_=bin_[:, :])
            wklo_t = pers.tile([128, K * PC_N], f32)
            nc.sync.dma_start(out=wklo_t, in_=wklo[:, :])
            wkhi_t = pers.tile([128, K * PC_N], f32)
            nc.sync.dma_start(out=wkhi_t, in_=wkhi[:, :])
            posm_t = [pers.tile([PR[pi], IJ], f16, name=f"posm{pi}") for pi in range(2)]
            for pi in range(2):
                nc.sync.dma_start(out=posm_t[pi], in_=posm_d[pi][:, :])

            # GRU context (H' = h+1), per h-chunk: [128, r*224]
            ctx = [pers.tile([128, R * BC], f16, name=f"ctx{t}") for t in range(2)]
            h0 = pers.tile([128, BC], f16)
            nc.vector.memset(h0, 1.0)

            predsT = [
                [
                    pers.tile([128, BS * PR[pi]], f16, name=f"pt{pi}_{m}")
                    for m in range(PC_N)
                ]
                for pi in range(2)
            ]
            cnt_t = [pers.tile([PR[pi], BS * IJ], f16, name=f"cnt{pi}") for pi in range(2)]
            corr_t = [pers.tile([PR[pi], BS * IJ], f16, name=f"corr{pi}") for pi in range(2)]
            D_t = [pers.tile([PR[pi], BS * IJ], f16, name=f"D{pi}") for pi in range(2)]
            Ssum = [pers.tile([PR[pi], 4], f32, name=f"Ssum{pi}") for pi in range(2)]
            ones_t = [pers.tile([PR[pi], 1], f32, name=f"ones{pi}") for pi in range(2)]
            for pi in range(2):
                nc.vector.memset(ones_t[pi], 1.0)
            outS = pers.tile([1, 8], f32)

            # scratch pools
            scr = tc.alloc_tile_pool(name="scr", bufs=1)
            wkp = tc.alloc_tile_pool(name="wkp", bufs=1)
            psPP = tc.alloc_tile_pool(name="psPP", bufs=3, space="PSUM")

            wk_t = {}

            def load_wk(k):
                t = wkp.tile([128, HC_N * P], f16, tag="wk", bufs=3, name=f"wk{k}")
                nc.sync.dma_start(out=t, in_=wk[k, :, :])
                wk_t[k] = t

            # ---------- preds emission ----------
            def emit_preds_chunk(pi, k, rs):
                nq = len(rs)
                n = nq * BC
                for m in range(PC_N):
                    ps = psPP.tile([128, 512], f32, tag="pp", name=f"pp{pi}_{k}_{rs[0]}_{m}")
                    for hc in range(HC_N):
                        nc.tensor.matmul(
                            ps[:, :n],
                            wk_t[k][:, hc * P + m * 128 : hc * P + (m + 1) * 128],
                            ctx[hc][:, rs[0] * BC : (rs[0] + nq) * BC],
                            start=(hc == 0),
                            stop=(hc == HC_N - 1),
                        )
                    off = ROW_OFF[(k, rs[0])][1]
                    dst = (
                        predsT[pi][m]
                        .rearrange("p (b x) -> p b x", b=BS)[:, :, off : off + nq * C]
                        .rearrange("p b (q c) -> p q b c", q=nq)
                    )
                    lo = wklo_t[:, k * PC_N + m : k * PC_N + m + 1]
                    hi = wkhi_t[:, k * PC_N + m : k * PC_N + m + 1]
                    psv = ps[:, :n].rearrange("p (q b c) -> p q b c", q=nq, b=BS)
                    if m % 2 == 0:
                        nc.vector.tensor_scalar(dst, psv, lo, hi, Alu.max, Alu.min)
                    else:
                        ev = scr.tile([128, 448], f16, tag="ev", bufs=3, name=f"ev{pi}{k}{rs[0]}{m}")
                        evs = ev[:, :n]
                        nc.scalar.activation(evs, ps[:, :n], Act.Identity)
                        nc.vector.tensor_scalar(
                            dst,
                            evs.rearrange("p (q b c) -> p q b c", q=nq, b=BS),
                            lo, hi, Alu.max, Alu.min,
                        )

            # ---- phase 1: GRU (fused gates) ----
            with (
                tc.tile_pool(name="p1", bufs=1) as p1,
                tc.tile_pool(name="psG", bufs=2, space="PSUM") as psG,
                tc.tile_pool(name="psH", bufs=2, space="PSUM") as psH,
            ):
                wih_b = p1.tile([128, PC_N * 768], f16, name="wih_b")
                nc.sync.dma_start(out=wih_b, in_=wih[:, :])
                whh_b = p1.tile([128, HC_N * 768], f16, name="whh_b")
                nc.sync.dma_start(out=whh_b, in_=whh[:, :])
                enc_b = p1.tile([128, R * PC_N * BC], f16, name="enc_b")
                for r in range(R):
                    sl = slice(r * PC_N * BC, (r + 1) * PC_N * BC)
                    nc.scalar.dma_start(out=enc_b[:, sl], in_=encT[:, sl])
                encv = enc_b.rearrange("p (r pc x) -> p r pc x", r=R, pc=PC_N)

                def wih_s(pc, m):
                    return wih_b[:, pc * 768 + m * 128 : pc * 768 + (m + 1) * 128]

                def whh_s(hc, m):
                    return whh_b[:, hc * 768 + m * 128 : hc * 768 + (m + 1) * 128]

                gin = [p1.tile([128, R * BC], f16, name=f"gin{t}") for t in range(2)]

                def emit_gin_chunk(t, ch):
                    # gi for the n gate, steps 2ch and 2ch+1 (448 cols)
                    ps = psPP.tile([128, 512], f32, tag="pp", name=f"gin{t}_{ch}")
                    for pc in range(PC_N):
                        nc.tensor.matmul(
                            ps[:, : 2 * BC],
                            wih_s(pc, 4 + t),
                            encv[:, 2 * ch : 2 * ch + 2, pc : pc + 1, :],
                            start=(pc == 0),
                            stop=(pc == PC_N - 1),
                        )
                    nc.scalar.activation(
                        gin[t][:, 2 * ch * BC : (2 * ch + 2) * BC],
                        ps[:, : 2 * BC],
                        Act.Identity,
                        bias=bin_t[:, t : t + 1],
                    )

                load_wk(4)
                load_wk(3)
                load_wk(2)
                for pi in range(2):
                    nc.sync.dma_start(out=cnt_t[pi], in_=cnt_d[pi][:, :])
                    nc.sync.dma_start(out=corr_t[pi], in_=corr_d[pi][:, :])
                load_wk(0)
                load_wk(1)

                def emit_gru_step(r):
                    hprev = [h0, h0] if r == 0 else [
                        ctx[t][:, (r - 1) * BC : r * BC] for t in range(2)
                    ]
                    gps = []
                    hps = []
                    for t in range(2):
                        ps = psG.tile([128, 448], f32, tag="g", name=f"g{r}_{t}")
                        for half, m in ((0, t), (1, 2 + t)):   # r gate, z gate
                            sl = ps[:, half * BC : (half + 1) * BC]
                            for pc in range(PC_N):
                                nc.tensor.matmul(
                                    sl, wih_s(pc, m),
                                    enc_b[:, (r * PC_N + pc) * BC : (r * PC_N + pc + 1) * BC],
                                    start=(pc == 0), stop=False,
                                )
                            for hc in range(HC_N):
                                nc.tensor.matmul(
                                    sl, whh_s(hc, m), hprev[hc],
                                    start=False, stop=(hc == HC_N - 1),
                                )
                        gps.append(ps)
                        ph = psH.tile([128, BC], f32, tag="h", name=f"h{r}_{t}")
                        for hc in range(HC_N):
                            nc.tensor.matmul(
                                ph, whh_s(hc, 4 + t), hprev[hc],
                                start=(hc == 0), stop=(hc == HC_N - 1),
                            )
                        hps.append(ph)
                    for t in range(2):
                        gr = gps[t][:, 0:BC]
                        gz = gps[t][:, BC : 2 * BC]
                        rt = scr.tile([128, BC], f16, tag="rt", bufs=2, name=f"rt{r}{t}")
                        nc.scalar.activation(rt, gr, Act.Sigmoid, bias=brz_t[:, t : t + 1])
                        zt = scr.tile([128, BC], f16, tag="zt", bufs=2, name=f"zt{r}{t}")
                        nc.scalar.activation(zt, gz, Act.Sigmoid, bias=brz_t[:, 2 + t : 3 + t])
                        z1 = scr.tile([128, BC], f16, tag="z1", bufs=2, name=f"z1{r}{t}")
                        nc.scalar.activation(
                            z1, gz, Act.Sigmoid, bias=nbrz_t[:, t : t + 1], scale=-1.0
                        )
                        hns = scr.tile([128, BC], f16, tag="hns", bufs=2, name=f"hns{r}{t}")
                        nc.scalar.activation(
                            hns, hps[t], Act.Identity, bias=bhn_t[:, t : t + 1]
                        )
                        tV = scr.tile([128, BC], f16, tag="tV", bufs=2, name=f"tV{r}{t}")
                        nc.vector.tensor_tensor(tV, hns, rt, op=Alu.mult)
                        tW = scr.tile([128, BC], f16, tag="tW", bufs=2, name=f"tW{r}{t}")
                        nc.vector.tensor_tensor(
                            tW, tV, gin[t][:, r * BC : (r + 1) * BC], op=Alu.add
                        )
                        sv = scr.tile([128, BC], f16, tag="sv", bufs=2, name=f"sv{r}{t}")
                        nc.scalar.activation(sv, tW, Act.Sigmoid, scale=2.0)
                        a_ = scr.tile([128, BC], f16, tag="a_", bufs=2, name=f"a{r}{t}")
                        nc.vector.tensor_tensor(a_, sv, z1, op=Alu.mult)
                        b2 = scr.tile([128, BC], f16, tag="b2", bufs=2, name=f"b{r}{t}")
                        nc.vector.tensor_tensor(b2, zt, hprev[t], op=Alu.mult)
                        nc.vector.scalar_tensor_tensor(
                            ctx[t][:, r * BC : (r + 1) * BC],
                            a_, 2.0, b2, op0=Alu.mult, op1=Alu.add,
                        )

                emit_gin_chunk(0, 0)
                emit_gin_chunk(1, 0)
                emit_gru_step(0)
                emit_gin_chunk(0, 1)
                emit_gin_chunk(1, 1)
                emit_gru_step(1)
                emit_gin_chunk(0, 2)
                emit_preds_chunk(0, 4, [0, 1])
                emit_gru_step(2)
                emit_gin_chunk(1, 2)
                emit_preds_chunk(0, 3, [0, 1])
                emit_preds_chunk(0, 3, [2])
                emit_gru_step(3)
                emit_preds_chunk(1, 1, [0, 1])
                emit_gru_step(4)
                emit_preds_chunk(0, 2, [0, 1])
                emit_preds_chunk(0, 2, [2, 3])
                emit_preds_chunk(1, 0, [0, 1])
                emit_gru_step(5)
                emit_preds_chunk(1, 0, [2, 3])
                emit_preds_chunk(1, 1, [2, 3])
                emit_preds_chunk(1, 1, [4])
                emit_preds_chunk(1, 0, [4, 5])

            # ---- phase 3: dots + loss ----
            with (
                tc.tile_pool(name="p3", bufs=1) as p3,
                tc.tile_pool(name="psDP", bufs=3, space="PSUM") as psDP,
            ):
                encB_b = p3.tile([128, PC_N * BS * IJ], f16, name="encB_b")
                hw = PC_N * BS * IJ // 2
                nc.scalar.dma_start(out=encB_b[:, :hw], in_=encB[:, :hw])
                nc.gpsimd.dma_start(out=encB_b[:, hw:], in_=encB[:, hw:])

                def emit_dots(pi, bb):
                    rows = PR[pi]
                    ps = psDP.tile([rows, 2 * IJ], f32, tag="dp", name=f"dp{pi}_{bb}")
                    for half in range(2):
                        b = 2 * bb + half
                        for pc in range(PC_N):
                            nc.tensor.matmul(
                                ps[:, half * IJ : (half + 1) * IJ],
                                predsT[pi][pc][:, b * rows : (b + 1) * rows],
                                encB_b[:, pc * BS * IJ + b * IJ : pc * BS * IJ + (b + 1) * IJ],
                                start=(pc == 0),
                                stop=(pc == PC_N - 1),
                            )
                    csl = slice(2 * bb * IJ, (2 * bb + 2) * IJ)
                    nc.vector.tensor_tensor(
                        D_t[pi][:, csl], ps, corr_t[pi][:, csl], op=Alu.add
                    )

                PG = BS // 2   # 16 groups per post part

                def emit_post(pi, h):
                    rows = PR[pi]
                    c0 = h * PG * IJ
                    Dp = D_t[pi][:, c0 : c0 + PG * IJ]
                    Dv = Dp.rearrange("p (g j) -> p g j", j=IJ)
                    mxg = scr.tile([rows, PG], f16, tag=f"mxg{pi}", bufs=2, name=f"mxg{pi}{h}")
                    nc.vector.tensor_reduce(mxg, Dv, axis=AxX, op=Alu.max)
                    mxp = scr.tile([rows, 1], f32, tag=f"mxp{pi}", bufs=2, name=f"mxp{pi}{h}")
                    nc.vector.tensor_reduce(mxp, mxg, axis=AxX, op=Alu.max)
                    nmx = scr.tile([rows, 1], f32, tag=f"nmx{pi}", bufs=2, name=f"nmx{pi}{h}")
                    nc.scalar.activation(nmx, mxp, Act.Identity, scale=-1.0)
                    B2 = p3.tile([rows, PG * IJ], f32, tag=f"B2{pi}", bufs=2, name=f"B2{pi}{h}")
                    nc.scalar.activation(B2, Dp, Act.Exp, bias=nmx[:, 0:1])
                    nc.vector.tensor_tensor(
                        B2, B2, cnt_t[pi][:, c0 : c0 + PG * IJ], op=Alu.mult
                    )
                    se = scr.tile([rows, PG], f32, tag=f"se{pi}", bufs=2, name=f"se{pi}{h}")
                    nc.vector.tensor_reduce(
                        se, B2.rearrange("p (g j) -> p g j", j=IJ), axis=AxX, op=Alu.add
                    )
                    P2 = p3.tile([rows, PG * IJ], f16, tag=f"P2{pi}", bufs=2, name=f"P2{pi}{h}")
                    nc.vector.tensor_tensor(
                        P2.rearrange("p (g j) -> p g j", j=IJ),
                        Dv,
                        posm_t[pi].unsqueeze(1).broadcast_to([rows, PG, IJ]),
                        op=Alu.mult,
                    )
                    pos = scr.tile([rows, PG], f16, tag=f"pos{pi}", bufs=2, name=f"pos{pi}{h}")
                    with nc.allow_low_precision(reason="1-hot sum, exact in fp16"):
                        nc.vector.tensor_reduce(
                            pos, P2.rearrange("p (g j) -> p g j", j=IJ), axis=AxX, op=Alu.add
                        )
                    lnse = scr.tile([rows, PG], f32, tag=f"ls{pi}", bufs=2, name=f"ls{pi}{h}")
                    nc.scalar.activation(lnse, se, Act.Ln)
                    lnv = scr.tile([rows, PG], f32, tag=f"lv{pi}", bufs=2, name=f"lv{pi}{h}")
                    nc.vector.scalar_tensor_tensor(
                        lnv, lnse, mxp[:, 0:1], pos, op0=Alu.add, op1=Alu.subtract
                    )
                    ch = scr.tile([rows, PG], f32, tag=f"ch{pi}", bufs=2, name=f"ch{pi}{h}")
                    nc.vector.tensor_tensor(ch, pos, mxg, op=Alu.is_ge)
                    nc.vector.tensor_reduce(
                        Ssum[pi][:, 2 * h : 2 * h + 1], lnv, axis=AxX, op=Alu.add
                    )
                    nc.vector.tensor_reduce(
                        Ssum[pi][:, 2 * h + 1 : 2 * h + 2], ch, axis=AxX, op=Alu.add
                    )

                for bb in range(BS // 2):
                    emit_dots(0, bb)
                emit_post(0, 0)
                for bb in range(BS // 2):
                    emit_dots(1, bb)
                emit_post(0, 1)
                emit_post(1, 0)
                emit_post(1, 1)

                for pi in range(2):
                    pf = psDP.tile([1, 4], f32, tag="fin", bufs=2, name=f"fin{pi}")
                    nc.tensor.matmul(pf, ones_t[pi], Ssum[pi], start=True, stop=True)
                    nc.vector.tensor_copy(outS[:, 4 * pi : 4 * pi + 4], pf)
                nc.sync.dma_start(out=out[:, :], in_=outS)
            psPP.release()
            wkp.release()
            scr.release()

    nc.finalize()
    return nc


def _prep_inputs(encodings, hidden, W_ih, W_hh, b_ih, b_hh, Wk_w, Wk_b,
                 neg_rows, neg_cols):
    """Host-side reformat of the full inputs into per-core DMA-clean arrays."""
    f16 = np.float16
    enc = np.ascontiguousarray(encodings, dtype=np.float32)
    e6 = enc.reshape(NCORE, BS, C, C, PC_N, 128)  # (core, b, i, c, pc, pp)
    # GRU layout (r-major): [core, pp, r, pc, b*7+c]
    encT = np.ascontiguousarray(
        e6[:, :, :R].transpose(0, 5, 2, 4, 1, 3)   # (core, pp, r, pc, b, c)
    ).reshape(NCORE, 128, R * PC_N * BC).astype(f16)
    # dots layout: [core, pp, pc, b*49 + i*7 + c]
    encB = np.ascontiguousarray(
        e6.transpose(0, 5, 4, 1, 2, 3)   # (core, pp, pc, b, i, c)
    ).reshape(NCORE, 128, PC_N * BS * IJ).astype(f16)

    wih = np.ascontiguousarray(
        W_ih.T.reshape(PC_N, 128, 768).transpose(1, 0, 2), dtype=np.float32
    ).reshape(128, PC_N * 768).astype(f16)
    whh = np.ascontiguousarray(
        W_hh.T.reshape(HC_N, 128, 768).transpose(1, 0, 2), dtype=np.float32
    ).reshape(128, HC_N * 768).astype(f16)
    wkh = np.ascontiguousarray(
        Wk_w.transpose(0, 2, 1).reshape(K, HC_N, 128, P).transpose(0, 2, 1, 3),
        dtype=np.float32,
    ).reshape(K, 128, HC_N * P).astype(f16)

    # H' = h + 1 bias folds
    rs = W_hh.astype(np.float32).sum(axis=1)              # [768]
    bsum = (b_ih + b_hh).astype(np.float32) - rs
    brz = np.ascontiguousarray(bsum[:512].reshape(4, 128).T)
    nbrz = np.ascontiguousarray((-bsum[256:512]).reshape(2, 128).T)
    bhn = np.ascontiguousarray(
        (b_hh.astype(np.float32) - rs)[512:].reshape(2, 128).T
    )
    bin_ = np.ascontiguousarray(b_ih[512:].astype(np.float32).reshape(2, 128).T)

    bias_k = Wk_b.astype(np.float32) - Wk_w.astype(np.float32).sum(axis=2)  # [K, P]
    wkbT = np.ascontiguousarray(
        bias_k.reshape(K, PC_N, 128).transpose(2, 0, 1)
    ).reshape(128, K * PC_N)
    wklo = -1.0 - wkbT
    wkhi = 1.0 - wkbT
    corr_k = np.einsum("kp,bijp->kbij", bias_k, enc, optimize=True).reshape(K, B, IJ)

    # negatives -> multiplicity counts over the 49 cells, plus the positive
    neg_idx = (neg_rows.astype(np.int64) * 7 + neg_cols.astype(np.int64))
    cnts = np.zeros((B, K, R, C, IJ), dtype=np.float32)
    np.add.at(
        cnts.reshape(B * K * R * C, IJ),
        (
            np.repeat(np.arange(B * K * R * C, dtype=np.int64), S - 1),
            neg_idx.reshape(-1),
        ),
        1.0,
    )
    cvec = np.arange(C)
    for k in range(K):
        for r in range(R):
            cnts[:, k, r, cvec, r * 7 + cvec] += 1.0   # the positive

    corr_dev, cnt_dev, posm_dev = [], [], []
    for pi, lst in enumerate(PASS_PAIRS):
        rows = PR[pi]
        cd = np.empty((NCORE, rows, BS, IJ), dtype=np.float32)
        nd = np.empty((NCORE, rows, BS, IJ), dtype=np.float32)
        pm = np.zeros((rows, IJ), dtype=np.float32)
        for qi, (k, r) in enumerate(lst):
            for c in range(C):
                row = qi * C + c
                cd[:, row] = corr_k[k].reshape(NCORE, BS, IJ)
                nd[:, row] = cnts[:, k, r, c].reshape(NCORE, BS, IJ)
                pm[row, r * 7 + c] = 1.0
        cd = cd + np.float32(MASK) * (nd == 0)
        corr_dev.append(cd.reshape(NCORE, rows, BS * IJ).astype(f16))
        cnt_dev.append(nd.reshape(NCORE, rows, BS * IJ).astype(f16))
        posm_dev.append(pm.astype(f16))

    in_maps = []
    for core in range(NCORE):
        in_maps.append(
            {
                "encT": encT[core],
                "encB": encB[core],
                "wih": wih,
                "whh": whh,
                "wk": wkh,
                "brz": brz,
                "nbrz": nbrz,
                "bhn": bhn,
                "bin": bin_,
                "wklo": wklo,
                "wkhi": wkhi,
                "corr0": corr_dev[0][core],
                "corr1": corr_dev[1][core],
                "cnt0": cnt_dev[0][core],
                "cnt1": cnt_dev[1][core],
                "posm0": posm_dev[0],
                "posm1": posm_dev[1],
            }
        )
    return in_maps


def _get_program():
    if "nc" not in _CACHE:
        _CACHE["nc"] = _build_program()
    return _CACHE["nc"]


def run_on_device(in_maps, trace=False, tmpdir=None):
    from concourse.bass_utils import run_bass_kernel_spmd

    nc = _get_program()
    return run_bass_kernel_spmd(
        nc, in_maps, list(range(NCORE)), trace=trace, tmpdir=tmpdir
    )


def kernel(**inputs):
    in_maps = _prep_inputs(**inputs)
    res = run_on_device(in_maps)
    loss_sum = 0.0
    corr_sum = 0.0
    for core in range(NCORE):
        o = np.asarray(res.results[core]["out"], dtype=np.float64).reshape(8)
        loss_sum += o[0] + o[2] + o[4] + o[6]
        corr_sum += o[1] + o[3] + o[5] + o[7]
    loss = np.float32(loss_sum / N_PREDS)
    acc = np.float32(corr_sum / N_PREDS)
    return loss, acc
